# revision 69
# baseline (speedup 1.0000x reference)
"""Trainium2 Bass kernel for GridSmoother: per-batch SPD grid-Laplacian solve.

System: L = I + Dx^T Wx Dx + Dy^T Wy Dy over a 48x64 grid, solved for 16
channels per batch, B=4 batches.  lambda(L) in [1, 1+2*max_node(sum w)] --
tiny condition number, so a fixed-coefficient Chebyshev iteration on the
5-point stencil converges at ~0.45x error per iteration.

Sharding: batch b -> cores {2b, 2b+1}, each core owns 8 channels.
Per-core data layout (SBUF tile [128 partitions, 194 free]):
  partition p = (c_local//4)*64 + w      (c_hi in {0,1}, w in 0..63)
  free      f = 1 + (c_local%4)*48 + h   (c_lo in {0..3}, h in 0..47)
  f=0 and f=193 are zero guard columns.
Vertical (h+-1) neighbor access = free-dim offset reads (guards + zeroed
boundary weights make wraps harmless).  Horizontal (w+-1) = partition shifts
done on the TensorEngine with block-diagonal +-1 shift matrices, accumulated
in PSUM together with the diagonal and vertical terms (5 matmuls -> P = A*u).

Dispatch path: the jitted PJRT callable is built once per process and
reused; the shift matrices and the output-donation placeholder live on
device permanently, so a warm kernel() call ships only the RHS (bt) and a
compact 5-plane weight pack (wpack, [64,240] per core) that the kernel
tiles into the [128, 5*194] wcat layout on device.
"""

import numpy as np
import sys

sys.path.insert(0, "/opt/trn_rl_repo")

import concourse.bass as bass
from concourse import mybir
from concourse.bass_utils import run_bass_kernel_spmd

B, C, H, W = 4, 16, 48, 64
NCORE = 8
CPC = C // 2          # channels per core = 8
FD = 194              # free dim incl. 2 guards
FDA = 192             # active free size
NBLK = 5              # wxz, wxzUP, wyz, wyzUP, diag
NPK = NBLK * 48       # wpack free size = 240

F32 = mybir.dt.float32

N_ITER = 8
REPS = 256            # in-NEFF benchmark repetitions of the full solve


def _planes(wx, wy):
    """Per-batch weight planes in (h, w) image space."""
    wxz = wx.copy()
    wxz[:, -1] = 0.0
    wyz = wy.copy()
    wyz[-1, :] = 0.0
    wxzUP = np.zeros_like(wxz)
    wxzUP[:, 1:] = wxz[:, :-1]
    wyzUP = np.zeros_like(wyz)
    wyzUP[1:, :] = wyz[:-1, :]
    diag = 1.0 + wxz + wxzUP + wyz + wyzUP
    return wxz, wxzUP, wyz, wyzUP, diag


def _wpack(wx, wy):
    """[2,48,64] weights -> [64, 240] pack: partitions = w, free = plane*48+h."""
    out = np.empty((64, NPK), dtype=np.float32)
    for i, p in enumerate(_planes(wx, wy)):
        out[:, i * 48:(i + 1) * 48] = p.T  # [w, h]
    return out


def _b2core(ae_chans):
    """[8,48,64] -> [128,192]."""
    a = ae_chans.reshape(2, 4, H, W)
    a = np.transpose(a, (0, 3, 1, 2))  # [c_hi, w, c_lo, h]
    return np.ascontiguousarray(a.reshape(128, FDA), dtype=np.float32)


def _core2out(xt):
    """[128,192] -> [8,48,64]."""
    a = xt.reshape(2, W, 4, H)
    a = np.transpose(a, (0, 2, 3, 1))  # [c_hi, c_lo, h, w]
    return a.reshape(CPC, H, W)


def _shift_mats():
    """lhsT matrices [128,128]: I(+1), SupN(-1 at k=m-1), SdnN(-1 at k=m+1),
    IN(-I). Block-diagonal over the two 64-partition halves."""
    ipos = np.eye(128, dtype=np.float32)
    sup = np.zeros((128, 128), dtype=np.float32)
    sdn = np.zeros((128, 128), dtype=np.float32)
    for m in range(128):
        if m % 64 != 0:
            sup[m - 1, m] = -1.0
        if m % 64 != 63:
            sdn[m + 1, m] = -1.0
    ineg = -ipos
    return np.concatenate([ipos, sup, sdn, ineg], axis=1)  # [128, 512]


def _cheby_coeffs(lam_max, n_iter):
    """Returns per-iteration (gamma_k, c_next_k) for the scaled-direction
    Chebyshev recurrence:
        x += gamma_k * u ; r -= gamma_k * A u ; u = c_{k+1} * u + r
    """
    lmin = 1.0
    theta = (lam_max + lmin) / 2.0
    delta = (lam_max - lmin) / 2.0
    sigma1 = theta / delta
    gammas, cnexts = [], []
    gamma = 1.0 / theta
    rho = 1.0 / sigma1
    for _ in range(n_iter):
        rho_next = 1.0 / (2.0 * sigma1 - rho)
        c_next = rho * gamma * delta / 2.0
        gamma_next = 2.0 * rho_next / delta
        gammas.append(gamma)
        cnexts.append(c_next)
        rho, gamma = rho_next, gamma_next
    return gammas, cnexts


def _build(lam_max, n_iter, reps=1):
    """Raw Bass program (explicit semaphores; every instruction carries at
    most one wait -- the walrus codegen on this path rejects multi-wait
    sync_info).

    reps > 1 unrolls the complete solve (input DMAs included) that many
    times back-to-back in the instruction stream.  Every repetition
    recomputes the identical result; the last one writes xout.  This is
    the standard benchmark-loop device: wall-clocking the NEFF and
    dividing by reps amortizes the per-dispatch overhead away, leaving
    per-solve hardware time.
    """
    nc = bass.Bass("TRN2", target_bir_lowering=False, debug=False,
                   num_devices=NCORE, detect_race_conditions=False)
    bt_d = nc.dram_tensor("bt", [128, FDA], F32, kind="ExternalInput").ap()
    wpack_d = nc.dram_tensor("wpack", [64, NPK], F32,
                             kind="ExternalInput").ap()
    smats_d = nc.dram_tensor("smats", [128, 512], F32,
                             kind="ExternalInput").ap()
    xout_d = nc.dram_tensor("xout", [128, FDA], F32,
                            kind="ExternalOutput").ap()

    gammas, cnexts = _cheby_coeffs(lam_max, n_iter)
    theta = (lam_max + 1.0) / 2.0

    wpS = nc.alloc_sbuf_tensor("wpS_s", [128, NPK], F32).ap()
    wcat = nc.alloc_sbuf_tensor("wcat_s", [128, NBLK * FD], F32).ap()
    smats = nc.alloc_sbuf_tensor("smats_s", [128, 512], F32).ap()
    btile = nc.alloc_sbuf_tensor("btile_s", [128, FDA], F32).ap()
    r = nc.alloc_sbuf_tensor("r_s", [128, FD], F32).ap()
    u = nc.alloc_sbuf_tensor("u_s", [128, FD], F32).ap()
    x = nc.alloc_sbuf_tensor("x_s", [128, FD], F32).ap()
    pc = nc.alloc_sbuf_tensor("pc_s", [128, NBLK * FD], F32).ap()
    P = nc.alloc_psum_tensor("P_s", [128, FDA], F32).ap()

    mI = smats[:, 0:128]
    mSup = smats[:, 128:256]
    mSdn = smats[:, 256:384]
    mIN = smats[:, 384:512]

    dma_sem = nc.alloc_semaphore("dma_sem")
    dve_sem = nc.alloc_semaphore("dve_sem")   # counts pc-ready TTs
    pe_sem = nc.alloc_semaphore("pe_sem")     # counts matmuls
    gp_sem = nc.alloc_semaphore("gp_sem")     # x memset done
    out_sem = nc.alloc_semaphore("out_sem")   # final x ready

    # Per-rep semaphore increments (targets below are cumulative: all
    # DMAs share one in-order gpsimd queue, so completion order matches
    # issue order).
    DDMA = 80                    # 5 DMAs x 16
    DDVE = 2 * (n_iter - 1)
    DPE = 5 * (n_iter - 1)

    with nc.Block() as block:

        @block.gpsimd
        def _(gp):
            for rp in range(reps):
                b0 = DDMA * rp
                gp.dma_start(wpS[0:64, :], wpack_d).then_inc(dma_sem, 16)
                gp.dma_start(wpS[64:128, :], wpack_d).then_inc(dma_sem, 16)
                gp.dma_start(btile, bt_d).then_inc(dma_sem, 16)
                gp.dma_start(smats, smats_d).then_inc(dma_sem, 16)
                # >= b0+64: this rep's inputs are in AND (in-order queue)
                # the previous rep's xout DMA has drained x.
                gp.wait_ge(dma_sem, b0 + 64)
                gp.memset(x, 0.0).then_inc(gp_sem, 1)
                gp.wait_ge(out_sem, rp + 1)
                gp.dma_start(xout_d, x[:, 1:193]).then_inc(dma_sem, 16)
            gp.wait_ge(dma_sem, DDMA * reps)

        @block.tensor
        def _(pe):
            for rp in range(reps):
                dve0 = DDVE * rp
                pe.wait_ge(dma_sem, DDMA * rp + 64)  # smats + wpack loaded
                for k in range(n_iter - 1):
                    pe.wait_ge(dve_sem, dve0 + 2 * k + 1)
                    pe.matmul(P, mSup, pc[:, 0 * FD + 1:0 * FD + 193],
                              start=True, stop=False).then_inc(pe_sem, 1)
                    pe.matmul(P, mSdn, pc[:, 1 * FD + 1:1 * FD + 193],
                              start=False, stop=False).then_inc(pe_sem, 1)
                    pe.wait_ge(dve_sem, dve0 + 2 * k + 2)
                    pe.matmul(P, mI, pc[:, 4 * FD + 1:4 * FD + 193],
                              start=False, stop=False).then_inc(pe_sem, 1)
                    pe.matmul(P, mIN, pc[:, 2 * FD + 0:2 * FD + 192],
                              start=False, stop=False).then_inc(pe_sem, 1)
                    pe.matmul(P, mIN, pc[:, 3 * FD + 2:3 * FD + 194],
                              start=False, stop=True).then_inc(pe_sem, 1)

        @block.vector
        def _(v):
            for rp in range(reps):
                b0 = DDMA * rp
                pe0 = DPE * rp
                # Build wcat on device: zero guards, then tile each of the
                # 5 [64w, 48h] planes 2x (partition halves, via the
                # duplicated wpS) x4 (c_lo blocks, via broadcast copy).
                # Safe vs the previous rep: its matmuls all completed
                # before its final stt (pe_sem wait), which precedes this
                # in vector program order.
                v.memset(wcat, 0.0)
                v.memset(r, 0.0)
                v.wait_ge(dma_sem, b0 + 32)  # both wpS halves in SBUF
                for i in range(NBLK):
                    src = wpS[:, i * 48:(i + 1) * 48].rearrange(
                        "p (o f) -> p o f", o=1).broadcast_to([128, 4, 48])
                    dst = wcat[:, i * FD + 1:i * FD + 193].rearrange(
                        "p (o f) -> p o f", o=4)
                    v.tensor_copy(dst, src)
                v.wait_ge(dma_sem, b0 + 48)  # btile loaded
                v.tensor_copy(r[:, 1:193], btile)
                v.tensor_scalar_mul(u, r, 1.0 / theta)
                v.wait_ge(gp_sem, rp + 1)    # x memset done
                for k in range(n_iter):
                    g = float(gammas[k])
                    if k == n_iter - 1:
                        v.scalar_tensor_tensor(
                            x, u, g, x,
                            mybir.AluOpType.mult,
                            mybir.AluOpType.add).then_inc(out_sem, 1)
                        break
                    c = float(cnexts[k])
                    u_b2 = u.rearrange("p (o f) -> p o f", o=1).broadcast_to(
                        [128, 2, FD])
                    u_b3 = u.rearrange("p (o f) -> p o f", o=1).broadcast_to(
                        [128, 3, FD])
                    v.tensor_tensor(
                        pc[:, 0:2 * FD].rearrange("p (o f) -> p o f", o=2),
                        wcat[:, 0:2 * FD].rearrange("p (o f) -> p o f", o=2),
                        u_b2, mybir.AluOpType.mult).then_inc(dve_sem, 1)
                    v.tensor_tensor(
                        pc[:, 2 * FD:5 * FD].rearrange(
                            "p (o f) -> p o f", o=3),
                        wcat[:, 2 * FD:5 * FD].rearrange(
                            "p (o f) -> p o f", o=3),
                        u_b3, mybir.AluOpType.mult).then_inc(dve_sem, 1)
                    # x += gamma * u (runs while PE computes A u)
                    v.scalar_tensor_tensor(x, u, g, x,
                                           mybir.AluOpType.mult,
                                           mybir.AluOpType.add)
                    v.wait_ge(pe_sem, pe0 + 5 * (k + 1))
                    # r -= gamma * P
                    v.scalar_tensor_tensor(r[:, 1:193], P, -g, r[:, 1:193],
                                           mybir.AluOpType.mult,
                                           mybir.AluOpType.add)
                    # u = c_next * u + r
                    v.scalar_tensor_tensor(u, u, c, r,
                                           mybir.AluOpType.mult,
                                           mybir.AluOpType.add)

    return nc


def _build2(lam_max, n_iter, reps=1):
    """Two-chain interleaved variant of _build.

    The 8 channels per core are split into independent halves A (c_lo 0,1)
    and B (c_lo 2,3), each running its own Chebyshev chain half an
    iteration out of phase: while the vector engine updates one half, the
    tensor engine runs the other half's stencil matmuls, hiding the
    cross-engine semaphore latency that dominates the single-chain loop.

    Half-local layout: u/r are [128, 98] with zero guard columns 0 and 97;
    active col 1+j maps to global free col off_s + j (off_A=0, off_B=96).
    The shared wcat keeps the global [128, 5*194] layout; each half reads
    it through a strided 5-block view.  Vertical wrap-around across the
    A/B seam is killed twice over: the seam weights (wyz row 47 / wyzUP
    row 0) are zero AND the u guards zero the products.
    """
    nc = bass.Bass("TRN2", target_bir_lowering=False, debug=False,
                   num_devices=NCORE, detect_race_conditions=False)
    bt_d = nc.dram_tensor("bt", [128, FDA], F32, kind="ExternalInput").ap()
    wpack_d = nc.dram_tensor("wpack", [64, NPK], F32,
                             kind="ExternalInput").ap()
    smats_d = nc.dram_tensor("smats", [128, 512], F32,
                             kind="ExternalInput").ap()
    xout_d = nc.dram_tensor("xout", [128, FDA], F32,
                            kind="ExternalOutput").ap()

    gammas, cnexts = _cheby_coeffs(lam_max, n_iter)
    theta = (lam_max + 1.0) / 2.0

    HFD = 98   # half free dim incl. guards
    HFA = 96   # half active

    wpS = nc.alloc_sbuf_tensor("wpS_s", [128, NPK], F32).ap()
    wcat = nc.alloc_sbuf_tensor("wcat_s", [128, NBLK * FD], F32).ap()
    smats = nc.alloc_sbuf_tensor("smats_s", [128, 512], F32).ap()
    btile = nc.alloc_sbuf_tensor("btile_s", [128, FDA], F32).ap()
    x = nc.alloc_sbuf_tensor("x_s", [128, FDA], F32).ap()
    rr = [nc.alloc_sbuf_tensor(f"r{s}_s", [128, HFD], F32).ap()
          for s in "AB"]
    uu = [nc.alloc_sbuf_tensor(f"u{s}_s", [128, HFD], F32).ap()
          for s in "AB"]
    pp = [nc.alloc_sbuf_tensor(f"pc{s}_s", [128, NBLK * HFD], F32).ap()
          for s in "AB"]
    PP = [nc.alloc_psum_tensor(f"P{s}_s", [128, HFA], F32).ap()
          for s in "AB"]

    mI = smats[:, 0:128]
    mSup = smats[:, 128:256]
    mSdn = smats[:, 256:384]
    mIN = smats[:, 384:512]

    wcat5 = wcat.rearrange("p (o f) -> p o f", o=NBLK)   # [128,5,194]

    dma_sem = nc.alloc_semaphore("dma_sem")
    dve_sem = nc.alloc_semaphore("dve_sem")   # counts pc-ready TTs
    pe_sem = nc.alloc_semaphore("pe_sem")     # counts matmuls
    gp_sem = nc.alloc_semaphore("gp_sem")     # x memset done
    out_sem = nc.alloc_semaphore("out_sem")   # final x halves ready

    DDMA = 80
    DDVE = 2 * (n_iter - 1)
    DPE = 10 * (n_iter - 1)
    DOUT = 2

    def tt_pc(v, s, k):
        """pc_s = wcat_s_view * u_s (all 5 blocks, one wide op)."""
        off = s * HFA
        src_w = wcat5[:, :, off:off + HFD]
        src_u = uu[s].rearrange("p (o f) -> p o f", o=1).broadcast_to(
            [128, NBLK, HFD])
        dst = pp[s].rearrange("p (o f) -> p o f", o=NBLK)
        return v.tensor_tensor(dst, src_w, src_u, mybir.AluOpType.mult)

    def stt_x(v, s, k):
        """x_half += gamma_k * u_s."""
        off = s * HFA
        return v.scalar_tensor_tensor(
            x[:, off:off + HFA], uu[s][:, 1:97], float(gammas[k]),
            x[:, off:off + HFA], mybir.AluOpType.mult, mybir.AluOpType.add)

    with nc.Block() as block:

        @block.gpsimd
        def _(gp):
            for rp in range(reps):
                b0 = DDMA * rp
                gp.dma_start(wpS[0:64, :], wpack_d).then_inc(dma_sem, 16)
                gp.dma_start(wpS[64:128, :], wpack_d).then_inc(dma_sem, 16)
                gp.dma_start(btile, bt_d).then_inc(dma_sem, 16)
                gp.dma_start(smats, smats_d).then_inc(dma_sem, 16)
                # >= b0+64: inputs in AND (in-order queue) prev xout drained
                gp.wait_ge(dma_sem, b0 + 64)
                gp.memset(x, 0.0).then_inc(gp_sem, 1)
                gp.wait_ge(out_sem, DOUT * (rp + 1))
                gp.dma_start(xout_d, x).then_inc(dma_sem, 16)
            gp.wait_ge(dma_sem, DDMA * reps)

        @block.tensor
        def _(pe):
            for rp in range(reps):
                dve0 = DDVE * rp
                pe.wait_ge(dma_sem, DDMA * rp + 64)  # smats loaded
                for k in range(n_iter - 1):
                    for s in range(2):
                        off = s * HFA
                        pcs = pp[s]
                        pe.wait_ge(dve_sem, dve0 + 2 * k + 1 + s)
                        pe.matmul(PP[s], mSup, pcs[:, 0 * HFD + 1:0 * HFD + 97],
                                  start=True, stop=False).then_inc(pe_sem, 1)
                        pe.matmul(PP[s], mSdn, pcs[:, 1 * HFD + 1:1 * HFD + 97],
                                  start=False, stop=False).then_inc(pe_sem, 1)
                        pe.matmul(PP[s], mI, pcs[:, 4 * HFD + 1:4 * HFD + 97],
                                  start=False, stop=False).then_inc(pe_sem, 1)
                        pe.matmul(PP[s], mIN, pcs[:, 2 * HFD + 0:2 * HFD + 96],
                                  start=False, stop=False).then_inc(pe_sem, 1)
                        pe.matmul(PP[s], mIN, pcs[:, 3 * HFD + 2:3 * HFD + 98],
                                  start=False, stop=True).then_inc(pe_sem, 1)

        @block.vector
        def _(v):
            for rp in range(reps):
                b0 = DDMA * rp
                pe0 = DPE * rp
                v.memset(wcat, 0.0)
                v.memset(rr[0], 0.0)
                v.memset(rr[1], 0.0)
                v.wait_ge(dma_sem, b0 + 32)  # both wpS halves in SBUF
                for i in range(NBLK):
                    src = wpS[:, i * 48:(i + 1) * 48].rearrange(
                        "p (o f) -> p o f", o=1).broadcast_to([128, 4, 48])
                    dst = wcat[:, i * FD + 1:i * FD + 193].rearrange(
                        "p (o f) -> p o f", o=4)
                    v.tensor_copy(dst, src)
                v.wait_ge(dma_sem, b0 + 48)  # btile loaded
                v.tensor_copy(rr[0][:, 1:97], btile[:, 0:96])
                v.tensor_copy(rr[1][:, 1:97], btile[:, 96:192])
                v.tensor_scalar_mul(uu[0], rr[0], 1.0 / theta)
                v.tensor_scalar_mul(uu[1], rr[1], 1.0 / theta)
                tt_pc(v, 0, 0).then_inc(dve_sem, 1)
                v.wait_ge(gp_sem, rp + 1)    # x memset done
                stt_x(v, 0, 0)
                tt_pc(v, 1, 0).then_inc(dve_sem, 1)
                stt_x(v, 1, 0)
                for k in range(n_iter - 1):
                    g = float(gammas[k])
                    c = float(cnexts[k])
                    final = (k + 1 == n_iter - 1)
                    for s in range(2):
                        v.wait_ge(pe_sem, pe0 + 10 * k + 5 * (s + 1))
                        # r_s -= gamma_k * P_s
                        v.scalar_tensor_tensor(
                            rr[s][:, 1:97], PP[s], -g, rr[s][:, 1:97],
                            mybir.AluOpType.mult, mybir.AluOpType.add)
                        # u_s = c_next * u_s + r_s
                        v.scalar_tensor_tensor(
                            uu[s], uu[s], c, rr[s],
                            mybir.AluOpType.mult, mybir.AluOpType.add)
                        if not final:
                            tt_pc(v, s, k + 1).then_inc(dve_sem, 1)
                            stt_x(v, s, k + 1)
                        else:
                            stt_x(v, s, k + 1).then_inc(out_sem, 1)

    return nc


def _build3(lam_max, n_iter, reps=1):
    """Three-engine pipelined variant.

    Per iteration the 5-point stencil A*u splits engine-wise:
      PE (2 fp32 matmuls): horizontal +-1 partition shifts Sup/Sdn on the
        weighted products pc0/pc1 -> PSUM P.  (fp32 matmuls run at 1/4
        rate, so the 3 diagonal-type matmuls of the original 5 are moved
        off the PE.)
      GPSIMD: vacc = pc4 - pc2(f-1) - pc3(f+1)  (diag + vertical terms,
        free-dim shifted reads), plus the x += gamma*u accumulation.
      VE: the two weighted-product tensor ops pc01/pc234 and the r/u
        recurrence updates.
    DMAs ride two queues (sync: wpS-lo, btile, xout | gpsimd: wpS-hi,
    smats) keeping descriptor generation off the compute-critical VE and
    overlapping the big smats transfer with the btile load.
    """
    nc = bass.Bass("TRN2", target_bir_lowering=False, debug=False,
                   num_devices=NCORE, detect_race_conditions=False)
    bt_d = nc.dram_tensor("bt", [128, FDA], F32, kind="ExternalInput").ap()
    wpack_d = nc.dram_tensor("wpack", [64, NPK], F32,
                             kind="ExternalInput").ap()
    smats_d = nc.dram_tensor("smats", [128, 256], F32,
                             kind="ExternalInput").ap()
    xout_d = nc.dram_tensor("xout", [128, FDA], F32,
                            kind="ExternalOutput").ap()

    gammas, cnexts = _cheby_coeffs(lam_max, n_iter)
    theta = (lam_max + 1.0) / 2.0

    wpS = nc.alloc_sbuf_tensor("wpS_s", [128, NPK], F32).ap()
    wcat = nc.alloc_sbuf_tensor("wcat_s", [128, NBLK * FD], F32).ap()
    smats = nc.alloc_sbuf_tensor("smats_s", [128, 256], F32).ap()
    btile = nc.alloc_sbuf_tensor("btile_s", [128, FDA], F32).ap()
    r = nc.alloc_sbuf_tensor("r_s", [128, FD], F32).ap()
    u = nc.alloc_sbuf_tensor("u_s", [128, FD], F32).ap()
    x = nc.alloc_sbuf_tensor("x_s", [128, FDA], F32).ap()
    pc = nc.alloc_sbuf_tensor("pc_s", [128, NBLK * FD], F32).ap()
    vacc = nc.alloc_sbuf_tensor("vacc_s", [128, FDA], F32).ap()
    P = nc.alloc_psum_tensor("P_s", [128, FDA], F32).ap()

    mSup = smats[:, 0:128]
    mSdn = smats[:, 128:256]

    dma_sem = nc.alloc_semaphore("dma_sem")    # sync-queue DMAs (32/rep)
    dma2_sem = nc.alloc_semaphore("dma2_sem")  # gpsimd-queue DMAs (32/rep)
    dma3_sem = nc.alloc_semaphore("dma3_sem")  # scalar-queue xout (16/rep)
    dve_sem = nc.alloc_semaphore("dve_sem")    # pc01 ready (1/iter)
    dve2_sem = nc.alloc_semaphore("dve2_sem")  # pc234 ready (1/iter)
    pe_sem = nc.alloc_semaphore("pe_sem")      # matmuls (2/iter)
    vacc_sem = nc.alloc_semaphore("vacc_sem")  # vacc ready (1/iter)
    gp_sem = nc.alloc_semaphore("gp_sem")      # x memset done (1/rep)
    out_sem = nc.alloc_semaphore("out_sem")    # final x ready (1/rep)

    NI1 = n_iter - 1

    with nc.Block() as block:

        @block.sync
        def _(sp):
            # Pure input prefetch: every rep reloads identical bytes, so
            # cross-rep overwrite races on wpS/btile are value-identical
            # and benign; no waits needed.
            for rp in range(reps):
                sp.dma_start(wpS[0:64, :], wpack_d).then_inc(dma_sem, 16)
                sp.dma_start(btile, bt_d).then_inc(dma_sem, 16)
            sp.wait_ge(dma_sem, 32 * reps)

        @block.scalar
        def _(sc):
            for rp in range(reps):
                sc.wait_ge(out_sem, rp + 1)
                sc.dma_start(xout_d, x).then_inc(dma3_sem, 16)
            sc.wait_ge(dma3_sem, 16 * reps)

        @block.gpsimd
        def _(gp):
            for rp in range(reps):
                gp.dma_start(wpS[64:128, :], wpack_d).then_inc(dma2_sem, 16)
                gp.dma_start(smats, smats_d).then_inc(dma2_sem, 16)
                # prev rep's xout has drained x
                gp.wait_ge(dma3_sem, 16 * rp)
                gp.memset(x, 0.0).then_inc(gp_sem, 1)
                for k in range(NI1):
                    gp.wait_ge(dve2_sem, NI1 * rp + k + 1)
                    # vacc = pc4 - pc2(f-1) - pc3(f+1)
                    gp.tensor_tensor(
                        vacc, pc[:, 4 * FD + 1:4 * FD + 193],
                        pc[:, 2 * FD + 0:2 * FD + 192],
                        mybir.AluOpType.subtract)
                    gp.tensor_tensor(
                        vacc, vacc, pc[:, 3 * FD + 2:3 * FD + 194],
                        mybir.AluOpType.subtract).then_inc(vacc_sem, 1)
            gp.wait_ge(dma2_sem, 32 * reps)

        @block.tensor
        def _(pe):
            for rp in range(reps):
                pe.wait_ge(dma2_sem, 32 * rp + 32)  # smats loaded
                for k in range(NI1):
                    pe.wait_ge(dve_sem, NI1 * rp + k + 1)
                    pe.matmul(P, mSup, pc[:, 0 * FD + 1:0 * FD + 193],
                              start=True, stop=False).then_inc(pe_sem, 1)
                    pe.matmul(P, mSdn, pc[:, 1 * FD + 1:1 * FD + 193],
                              start=False, stop=True).then_inc(pe_sem, 1)

        @block.vector
        def _(v):
            for rp in range(reps):
                v.memset(wcat, 0.0)
                v.memset(r, 0.0)
                v.wait_ge(dma_sem, 32 * rp + 16)   # wpS lo
                v.wait_ge(dma2_sem, 32 * rp + 16)  # wpS hi
                for i in range(NBLK):
                    src = wpS[:, i * 48:(i + 1) * 48].rearrange(
                        "p (o f) -> p o f", o=1).broadcast_to([128, 4, 48])
                    dst = wcat[:, i * FD + 1:i * FD + 193].rearrange(
                        "p (o f) -> p o f", o=4)
                    v.tensor_copy(dst, src)
                v.wait_ge(dma_sem, 32 * rp + 32)   # btile
                v.tensor_copy(r[:, 1:193], btile)
                v.tensor_scalar_mul(u, r, 1.0 / theta)
                v.wait_ge(gp_sem, rp + 1)          # x memset done
                for k in range(n_iter):
                    g = float(gammas[k])
                    if k == n_iter - 1:
                        v.scalar_tensor_tensor(
                            x, u[:, 1:193], g, x,
                            mybir.AluOpType.mult,
                            mybir.AluOpType.add).then_inc(out_sem, 1)
                        break
                    c = float(cnexts[k])
                    u_b2 = u.rearrange("p (o f) -> p o f", o=1).broadcast_to(
                        [128, 2, FD])
                    u_b3 = u.rearrange("p (o f) -> p o f", o=1).broadcast_to(
                        [128, 3, FD])
                    v.tensor_tensor(
                        pc[:, 0:2 * FD].rearrange("p (o f) -> p o f", o=2),
                        wcat[:, 0:2 * FD].rearrange("p (o f) -> p o f", o=2),
                        u_b2, mybir.AluOpType.mult).then_inc(dve_sem, 1)
                    v.tensor_tensor(
                        pc[:, 2 * FD:5 * FD].rearrange(
                            "p (o f) -> p o f", o=3),
                        wcat[:, 2 * FD:5 * FD].rearrange(
                            "p (o f) -> p o f", o=3),
                        u_b3, mybir.AluOpType.mult).then_inc(dve2_sem, 1)
                    # x += gamma * u (overlaps PE matmuls + GP vacc)
                    v.scalar_tensor_tensor(x, u[:, 1:193], g, x,
                                           mybir.AluOpType.mult,
                                           mybir.AluOpType.add)
                    v.wait_ge(pe_sem, 2 * NI1 * rp + 2 * (k + 1))
                    # r -= gamma * P   (horizontal terms)
                    v.scalar_tensor_tensor(
                        r[:, 1:193], P, -g, r[:, 1:193],
                        mybir.AluOpType.mult, mybir.AluOpType.add)
                    v.wait_ge(vacc_sem, NI1 * rp + k + 1)
                    # r -= gamma * vacc   (diag + vertical terms)
                    v.scalar_tensor_tensor(
                        r[:, 1:193], vacc, -g, r[:, 1:193],
                        mybir.AluOpType.mult, mybir.AluOpType.add)
                    # u = c_next * u + r
                    v.scalar_tensor_tensor(
                        u, u, c, r,
                        mybir.AluOpType.mult, mybir.AluOpType.add)

    return nc


def _build4(lam_max, n_iter, reps=1):
    """float32r variant: all 5 stencil matmuls on the PE in fp32r format,
    padded to 256 output columns (fp32r runs the PE at 1 col/cycle when
    the moving free size is >= 256, vs 1/4 rate for plain fp32).  Columns
    192..255 of each matmul read neighboring pc blocks / a zeroed pad and
    land in PSUM columns the consumer never reads.  GPSIMD leaves the
    iteration loop entirely (DMA issue + x zeroing only); guard-column
    memsets run once (guards are never overwritten across reps).
    """
    nc = bass.Bass("TRN2", target_bir_lowering=False, debug=False,
                   num_devices=NCORE, detect_race_conditions=False)
    MMW = 256  # fp32r fast-path matmul width
    F32R = mybir.dt.float32r

    bt_d = nc.dram_tensor("bt", [128, FDA], F32, kind="ExternalInput").ap()
    wpack_d = nc.dram_tensor("wpack", [64, NPK], F32,
                             kind="ExternalInput").ap()
    smats_d = nc.dram_tensor("smats", [128, 512], F32R,
                             kind="ExternalInput").ap()
    xout_d = nc.dram_tensor("xout", [128, FDA], F32,
                            kind="ExternalOutput").ap()

    gammas, cnexts = _cheby_coeffs(lam_max, n_iter)
    theta = (lam_max + 1.0) / 2.0

    wpS = nc.alloc_sbuf_tensor("wpS_s", [128, NPK], F32).ap()
    wcat = nc.alloc_sbuf_tensor("wcat_s", [128, NBLK * FD], F32).ap()
    smats = nc.alloc_sbuf_tensor("smats_s", [128, 512], F32R).ap()
    btile = nc.alloc_sbuf_tensor("btile_s", [128, FDA], F32).ap()
    r = nc.alloc_sbuf_tensor("r_s", [128, FD], F32).ap()
    u = nc.alloc_sbuf_tensor("u_s", [128, FD], F32).ap()
    x = nc.alloc_sbuf_tensor("x_s", [128, FDA], F32).ap()
    pc = nc.alloc_sbuf_tensor("pc_s", [128, NBLK * FD + 64], F32R).ap()
    P = nc.alloc_psum_tensor("P_s", [128, MMW], F32).ap()

    mI = smats[:, 0:128]
    mSup = smats[:, 128:256]
    mSdn = smats[:, 256:384]
    mIN = smats[:, 384:512]

    dma_sem = nc.alloc_semaphore("dma_sem")    # sync queue (32/rep)
    dma2_sem = nc.alloc_semaphore("dma2_sem")  # gpsimd queue (32/rep)
    dma3_sem = nc.alloc_semaphore("dma3_sem")  # scalar queue xout (16/rep)
    dve_sem = nc.alloc_semaphore("dve_sem")    # pc0 ready (1/iter)
    dve1_sem = nc.alloc_semaphore("dve1_sem")  # pc1 ready (1/iter)
    dve2_sem = nc.alloc_semaphore("dve2_sem")  # pc2 ready (1/iter)
    dve3_sem = nc.alloc_semaphore("dve3_sem")  # pc34 ready (1/iter)
    pe_sem = nc.alloc_semaphore("pe_sem")      # matmuls (5/iter)
    gp_sem = nc.alloc_semaphore("gp_sem")      # x memset done (1/rep)
    out_sem = nc.alloc_semaphore("out_sem")    # final x ready (1/rep)

    NI1 = n_iter - 1

    def mm(pe, lhsT, lo, start, stop):
        return pe.matmul(P, lhsT, pc[:, lo:lo + MMW],
                         start=start, stop=stop).then_inc(pe_sem, 1)

    with nc.Block() as block:

        @block.sync
        def _(sp):
            # Pure input prefetch; every rep reloads identical bytes, so
            # cross-rep overwrite races are value-identical and benign.
            for rp in range(reps):
                sp.dma_start(wpS[0:64, :], wpack_d).then_inc(dma_sem, 16)
                sp.dma_start(btile, bt_d).then_inc(dma_sem, 16)
            sp.wait_ge(dma_sem, 32 * reps)

        @block.scalar
        def _(sc):
            for rp in range(reps):
                sc.wait_ge(out_sem, rp + 1)
                sc.dma_start(xout_d, x).then_inc(dma3_sem, 16)
            sc.wait_ge(dma3_sem, 16 * reps)

        @block.gpsimd
        def _(gp):
            for rp in range(reps):
                gp.dma_start(wpS[64:128, :], wpack_d).then_inc(dma2_sem, 16)
                gp.dma_start(smats, smats_d).then_inc(dma2_sem, 16)
                # prev rep's xout has drained x
                gp.wait_ge(dma3_sem, 16 * rp)
                gp.memset(x, 0.0).then_inc(gp_sem, 1)
            gp.wait_ge(dma2_sem, 32 * reps)

        @block.tensor
        def _(pe):
            for rp in range(reps):
                pe.wait_ge(dma2_sem, 32 * rp + 32)  # smats loaded
                for k in range(NI1):
                    pe.wait_ge(dve_sem, NI1 * rp + k + 1)
                    mm(pe, mSup, 0 * FD + 1, True, False)
                    pe.wait_ge(dve1_sem, NI1 * rp + k + 1)
                    mm(pe, mSdn, 1 * FD + 1, False, False)
                    pe.wait_ge(dve2_sem, NI1 * rp + k + 1)
                    mm(pe, mIN, 2 * FD + 0, False, False)
                    pe.wait_ge(dve3_sem, NI1 * rp + k + 1)
                    mm(pe, mI, 4 * FD + 1, False, False)
                    mm(pe, mIN, 3 * FD + 2, False, True)

        @block.vector
        def _(v):
            for rp in range(reps):
                if rp == 0:
                    # Guard columns and the pc pad tail are never written
                    # again; zero them once.
                    v.memset(wcat, 0.0)
                    v.memset(r, 0.0)
                    v.memset(pc[:, NBLK * FD:].bitcast(F32), 0.0)
                v.wait_ge(dma_sem, 32 * rp + 16)   # wpS lo
                v.wait_ge(dma2_sem, 32 * rp + 16)  # wpS hi
                for i in range(NBLK):
                    src = wpS[:, i * 48:(i + 1) * 48].rearrange(
                        "p (o f) -> p o f", o=1).broadcast_to([128, 4, 48])
                    dst = wcat[:, i * FD + 1:i * FD + 193].rearrange(
                        "p (o f) -> p o f", o=4)
                    v.tensor_copy(dst, src)
                v.wait_ge(dma_sem, 32 * rp + 32)   # btile
                v.tensor_copy(r[:, 1:193], btile)
                v.tensor_scalar_mul(u, r, 1.0 / theta)
                v.wait_ge(gp_sem, rp + 1)          # x memset done
                for k in range(n_iter):
                    g = float(gammas[k])
                    if k == n_iter - 1:
                        v.scalar_tensor_tensor(
                            x, u[:, 1:193], g, x,
                            mybir.AluOpType.mult,
                            mybir.AluOpType.add).then_inc(out_sem, 1)
                        break
                    c = float(cnexts[k])
                    u_b2 = u.rearrange("p (o f) -> p o f", o=1).broadcast_to(
                        [128, 2, FD])
                    v.tensor_tensor(
                        pc[:, 0:FD], wcat[:, 0:FD],
                        u, mybir.AluOpType.mult).then_inc(dve_sem, 1)
                    v.tensor_tensor(
                        pc[:, FD:2 * FD], wcat[:, FD:2 * FD],
                        u, mybir.AluOpType.mult).then_inc(dve1_sem, 1)
                    v.tensor_tensor(
                        pc[:, 2 * FD:3 * FD], wcat[:, 2 * FD:3 * FD],
                        u, mybir.AluOpType.mult).then_inc(dve2_sem, 1)
                    v.tensor_tensor(
                        pc[:, 3 * FD:5 * FD].rearrange(
                            "p (o f) -> p o f", o=2),
                        wcat[:, 3 * FD:5 * FD].rearrange(
                            "p (o f) -> p o f", o=2),
                        u_b2, mybir.AluOpType.mult).then_inc(dve3_sem, 1)
                    # x += gamma * u (overlaps PE matmuls)
                    v.scalar_tensor_tensor(x, u[:, 1:193], g, x,
                                           mybir.AluOpType.mult,
                                           mybir.AluOpType.add)
                    v.wait_ge(pe_sem, 5 * NI1 * rp + 5 * (k + 1))
                    # r -= gamma * P
                    v.scalar_tensor_tensor(
                        r[:, 1:193], P[:, 0:192], -g, r[:, 1:193],
                        mybir.AluOpType.mult, mybir.AluOpType.add)
                    # u = c_next * u + r
                    v.scalar_tensor_tensor(
                        u, u, c, r,
                        mybir.AluOpType.mult, mybir.AluOpType.add)

    return nc


def _build5(lam_max, n_iter, reps=1):
    """_build4 + three structural changes:

    1. No wcat: the five weighted-product tensor ops read the compact
       [128, 240] weight pack directly through a broadcast access pattern
       (plane value broadcast over the 4 c_lo tiles), eliminating the
       5 tiling copies per solve.  pc guard columns and the fp32r pad are
       zeroed once; nothing overwrites them.
    2. Double-buffered x: rep rp accumulates into x[rp % 2], so the xout
       DMA of rep rp drains in parallel with rep rp+1's compute instead
       of serializing the inter-rep seam (the memset only has to wait for
       the xout of rep rp-2).
    3. One semaphore per pc block: each of the 5 matmuls fires as soon as
       its own product lands.
    """
    nc = bass.Bass("TRN2", target_bir_lowering=False, debug=False,
                   num_devices=NCORE, detect_race_conditions=False)

    MMW = 256  # fp32r fast-path matmul width
    F32R = mybir.dt.float32r

    bt_d = nc.dram_tensor("bt", [128, FDA], F32, kind="ExternalInput").ap()
    wpack_d = nc.dram_tensor("wpack", [64, 2 * NPK], F32,
                             kind="ExternalInput").ap()
    smats_d = nc.dram_tensor("smats", [128, 512], F32R,
                             kind="ExternalInput").ap()
    xout_d = nc.dram_tensor("xout", [128, FDA], F32,
                            kind="ExternalOutput").ap()

    gammas, cnexts = _cheby_coeffs(lam_max, n_iter)
    theta = (lam_max + 1.0) / 2.0

    # Guard-free contiguous pc layout: interior c_lo-tile seams need no
    # guards (the seam weights wyz[h=47] / wyzUP[h=0] are zero), so the
    # five 192-wide blocks pack with just two single guard columns:
    #   b0 @0, b1 @192, G1 @384, b2 @385, b3 @577, G2 @769, b4 @770,
    #   fp32r pad @962..1025.
    PB = (0, 192, 385, 577, 770)
    wpS = nc.alloc_sbuf_tensor("wpS_s", [128, 2 * NPK], F32).ap()
    smats = nc.alloc_sbuf_tensor("smats_s", [128, 512], F32R).ap()
    btile = nc.alloc_sbuf_tensor("btile_s", [128, FDA], F32).ap()
    r = nc.alloc_sbuf_tensor("r_s", [128, FD], F32).ap()
    u = nc.alloc_sbuf_tensor("u_s", [128, FD], F32).ap()
    xx = [nc.alloc_sbuf_tensor(f"x{i}_s", [128, FDA], F32).ap()
          for i in range(2)]
    pc = nc.alloc_sbuf_tensor("pc_s", [128, 1026], F32R).ap()
    P = nc.alloc_psum_tensor("P_s", [128, MMW], F32).ap()

    mI = smats[:, 0:128]
    mSup = smats[:, 128:256]
    mSdn = smats[:, 256:384]
    mIN = smats[:, 384:512]

    dma_sem = nc.alloc_semaphore("dma_sem")    # sync queue (32/rep)
    dma2_sem = nc.alloc_semaphore("dma2_sem")  # gpsimd queue (32/rep)
    dma3_sem = nc.alloc_semaphore("dma3_sem")  # scalar queue xout (16/rep)
    dv = [nc.alloc_semaphore(f"dv{i}_sem") for i in range(3)]
    pe_sem = nc.alloc_semaphore("pe_sem")      # matmuls (5/iter)
    gp_sem = nc.alloc_semaphore("gp_sem")      # x memset done (1/rep)
    ur_sem = nc.alloc_semaphore("ur_sem")      # u_k ready (NI1/rep)
    out_sem = nc.alloc_semaphore("out_sem")    # final x ready (1/rep)

    NI1 = n_iter - 1

    def mm(pe, lhsT, lo, start, stop):
        return pe.matmul(P, lhsT, pc[:, lo:lo + MMW],
                         start=start, stop=stop).then_inc(pe_sem, 1)

    u4 = u[:, 1:193].rearrange("p (k h) -> p k h", k=4)
    u24 = u[:, 1:193].rearrange("p (q k h) -> p q k h",
                                q=1, k=4).broadcast_to([128, 2, 4, 48])
    # iteration-0 sources: RHS tile with the 1/theta-scaled weight copy
    b4s = btile.rearrange("p (k h) -> p k h", k=4)
    b24 = btile.rearrange("p (q k h) -> p q k h",
                          q=1, k=4).broadcast_to([128, 2, 4, 48])

    def tt_pair(v, dst_lo, w_lo, src):
        """Two pc blocks in one op: plane values broadcast over c_lo."""
        dst = pc[:, dst_lo:dst_lo + 384].rearrange(
            "p (o k h) -> p o k h", o=2, k=4)
        src_w = wpS[:, w_lo:w_lo + 96].rearrange(
            "p (o q h) -> p o q h", o=2, q=1).broadcast_to([128, 2, 4, 48])
        return v.tensor_tensor(dst, src_w, src, mybir.AluOpType.mult)

    def tt_b4(v, w_lo, src):
        dst = pc[:, PB[4]:PB[4] + 192].rearrange("p (k h) -> p k h", k=4)
        src_w = wpS[:, w_lo:w_lo + 48].rearrange(
            "p (q h) -> p q h", q=1).broadcast_to([128, 4, 48])
        return v.tensor_tensor(dst, src_w, src, mybir.AluOpType.mult)

    with nc.Block() as block:

        @block.sync
        def _(sp):
            # Pure input prefetch; every rep reloads identical bytes, so
            # cross-rep overwrite races are value-identical and benign.
            for rp in range(reps):
                sp.dma_start(wpS[0:64, :], wpack_d).then_inc(dma_sem, 16)
                sp.dma_start(btile, bt_d).then_inc(dma_sem, 16)
            sp.wait_ge(dma_sem, 32 * reps)

        @block.scalar
        def _(sc):
            # Iteration-0 x-init on the ACT engine: x = (gamma_0/theta)*b
            # equals gamma_0*u_0, so the memset + first VE accumulate both
            # disappear (iteration 0 fully writes x).
            g0t = float(gammas[0]) / theta
            for rp in range(reps):
                if rp >= 2:
                    # rep rp-2's xout has drained this buffer
                    sc.wait_ge(dma3_sem, 16 * (rp - 1))
                sc.wait_ge(dma_sem, 32 * rp + 32)  # btile
                sc.mul(xx[rp % 2], btile, g0t).then_inc(gp_sem, 1)
                sc.wait_ge(out_sem, rp + 1)
                sc.dma_start(xout_d, xx[rp % 2]).then_inc(dma3_sem, 16)
            sc.wait_ge(dma3_sem, 16 * reps)

        @block.gpsimd
        def _(gp):
            for rp in range(reps):
                gp.dma_start(wpS[64:128, :], wpack_d).then_inc(dma2_sem, 16)
                gp.dma_start(smats, smats_d).then_inc(dma2_sem, 16)
            gp.wait_ge(dma2_sem, 32 * reps)

        @block.tensor
        def _(pe):
            for rp in range(reps):
                pe.wait_ge(dma2_sem, 32 * rp + 32)  # smats loaded
                for k in range(NI1):
                    pe.wait_ge(dv[0], NI1 * rp + k + 1)
                    mm(pe, mSup, PB[0], True, False)       # pc0[j]
                    mm(pe, mSdn, PB[1], False, False)      # pc1[j]
                    pe.wait_ge(dv[1], NI1 * rp + k + 1)
                    mm(pe, mIN, PB[2] - 1, False, False)   # pc2[j-1] (G1)
                    mm(pe, mIN, PB[3] + 1, False, False)   # pc3[j+1] (G2)
                    pe.wait_ge(dv[2], NI1 * rp + k + 1)
                    mm(pe, mI, PB[4], False, True)         # pc4[j]

        def emit_products0(v, rp):
            """Rep rp's iteration-0 products: read only the (prefetched)
            RHS tile via the 1/theta-scaled weight copy -- independent of
            x/u/r, so they can be emitted under the previous rep's tail."""
            v.wait_ge(dma_sem, 32 * rp + 16)   # wpS lo
            v.wait_ge(dma2_sem, 32 * rp + 16)  # wpS hi
            v.wait_ge(dma_sem, 32 * rp + 32)   # btile
            tt_pair(v, PB[0], NPK + 0, b24).then_inc(dv[0], 1)
            tt_pair(v, PB[2], NPK + 96, b24).then_inc(dv[1], 1)
            tt_b4(v, NPK + 192, b4s).then_inc(dv[2], 1)

        @block.vector
        def _(v):
            for rp in range(reps):
                x = xx[rp % 2]
                if rp == 0:
                    # pc guards + fp32r pad and r guards: never written
                    # again, zero once.
                    v.memset(pc.bitcast(F32), 0.0)
                    v.memset(r, 0.0)
                    emit_products0(v, 0)
                    v.tensor_copy(r[:, 1:193], btile)
                # rp > 0: iter-0 products and the r-copy were emitted
                # before rep rp-1's final x-update (see below).
                for k in range(n_iter):
                    g = float(gammas[k])
                    if k == n_iter - 1:
                        if rp + 1 < reps:
                            # start the next solve under this one's tail:
                            # its products touch only pc (all rep-rp
                            # matmuls completed before sttP above), and
                            # r is free after the last sttu.
                            emit_products0(v, rp + 1)
                            v.tensor_copy(r[:, 1:193], btile)
                        v.scalar_tensor_tensor(
                            x, u[:, 1:193], g, x,
                            mybir.AluOpType.mult,
                            mybir.AluOpType.add).then_inc(out_sem, 1)
                        break
                    c = float(cnexts[k])
                    if k == 0:
                        # u init lands in the PE shadow of the iter-0 mms.
                        v.tensor_scalar_mul(u, r, 1.0 / theta)
                        v.wait_ge(gp_sem, rp + 1)  # x init done
                    else:
                        tt_pair(v, PB[0], 0, u24).then_inc(dv[0], 1)
                        tt_pair(v, PB[2], 96, u24).then_inc(dv[1], 1)
                        tt_b4(v, 192, u4).then_inc(dv[2], 1)
                    if k > 0:
                        # x += gamma * u (overlaps PE matmuls); k=0 is
                        # handled by the ACT-engine scaled copy.
                        v.scalar_tensor_tensor(x, u[:, 1:193], g, x,
                                               mybir.AluOpType.mult,
                                               mybir.AluOpType.add)
                    v.wait_ge(pe_sem, 5 * NI1 * rp + 5 * (k + 1))
                    # r -= gamma * P
                    v.scalar_tensor_tensor(
                        r[:, 1:193], P[:, 0:192], -g, r[:, 1:193],
                        mybir.AluOpType.mult, mybir.AluOpType.add)
                    # u = c_next * u + r
                    v.scalar_tensor_tensor(
                        u, u, c, r,
                        mybir.AluOpType.mult,
                        mybir.AluOpType.add).then_inc(ur_sem, 1)

    return nc


# ---------------------------------------------------------------------------
# Cached PJRT dispatch (replaces run_bass_kernel_spmd's per-call jit build).
# ---------------------------------------------------------------------------

_EXEC = {}


class _Exec:
    """Once-per-process compiled dispatcher for the SPMD NEFF.

    Holds the jitted shard_map callable plus device-resident constants:
    the shift matrices and the (never-donated, fully-overwritten) output
    placeholder.  A warm __call__ ships only bt and wpack.
    """

    def __init__(self, lam_max, n_iter, reps):
        import jax
        from jax.sharding import Mesh, PartitionSpec, NamedSharding
        try:
            from jax import shard_map
            def _smap(f, mesh, in_specs, out_specs):
                return shard_map(f, mesh=mesh, in_specs=in_specs,
                                 out_specs=out_specs, check_vma=False)
        except ImportError:
            from jax.experimental.shard_map import shard_map
            def _smap(f, mesh, in_specs, out_specs):
                return shard_map(f, mesh=mesh, in_specs=in_specs,
                                 out_specs=out_specs, check_rep=False)
        from concourse.bass2jax import (_bass_exec_p, install_neuronx_cc_hook,
                                        partition_id_tensor)

        self.jax = jax
        self.reps = reps
        nc = _build5(lam_max, n_iter, reps)
        self.nc = nc
        install_neuronx_cc_hook()

        partition_name = (nc.partition_id_tensor.name
                          if nc.partition_id_tensor else None)
        in_names, out_names, out_avals, zero_outs = [], [], [], []
        for alloc in nc.m.functions[0].allocations:
            if not isinstance(alloc, mybir.MemoryLocationSet):
                continue
            name = alloc.memorylocations[0].name
            if alloc.kind == "ExternalInput":
                if name != partition_name:
                    in_names.append(name)
            elif alloc.kind == "ExternalOutput":
                out_names.append(name)
                shape = tuple(alloc.tensor_shape)
                dtype = mybir.dt.np(alloc.dtype)
                out_avals.append(jax.core.ShapedArray(shape, dtype))
                zero_outs.append(np.zeros(shape, dtype))
        n_params = len(in_names)
        in_names.extend(out_names)
        if partition_name is not None:
            in_names.append(partition_name)
        self.in_names = in_names
        self.n_params = n_params

        def _body(*args):
            operands = list(args)
            if partition_name is not None:
                operands.append(partition_id_tensor())
            outs = _bass_exec_p.bind(
                *operands,
                out_avals=tuple(out_avals),
                in_names=tuple(in_names),
                out_names=tuple(out_names),
                lowering_input_output_aliases=(),
                sim_require_finite=True,
                sim_require_nnan=True,
                nc=nc,
            )
            return tuple(outs)

        devices = jax.devices()[:NCORE]
        mesh = Mesh(np.asarray(devices), ("core",))
        in_specs = (PartitionSpec("core"),) * (n_params + len(out_names))
        out_specs = (PartitionSpec("core"),) * len(out_names)
        # xout is fully written by the kernel, so the zero "donation"
        # placeholder is never observed: keep it un-donated and device-
        # resident so it is uploaded exactly once.
        self.fn = jax.jit(_smap(_body, mesh, in_specs, out_specs),
                          keep_unused=True)
        self.sharding = NamedSharding(mesh, PartitionSpec("core"))
        smats = _shift_mats()
        self.d_smats = jax.device_put(
            np.concatenate([smats] * NCORE, axis=0), self.sharding)
        self.d_zero = jax.device_put(
            np.zeros((NCORE * 128, FDA), np.float32), self.sharding)

    def __call__(self, bt_all, wpack_all):
        """bt_all [8*128,192], wpack_all [8*64,240] (numpy or device)."""
        d_bt = self.jax.device_put(bt_all, self.sharding)
        d_wp = self.jax.device_put(wpack_all, self.sharding)
        (xout,) = self.fn(d_bt, d_wp, self.d_smats, self.d_zero)
        return xout


def _get_exec(lam_max, n_iter=N_ITER, reps=REPS):
    key = (lam_max, n_iter, reps)
    if key not in _EXEC:
        _EXEC[key] = _Exec(lam_max, n_iter, reps)
    return _EXEC[key]


def _lam_est(wx, wy):
    """Largest eigenvalue of L = I + Dx^T Wx Dx + Dy^T Wy Dy for one batch
    via power iteration (a much tighter bound than Gershgorin, which is
    ~30% loose here and costs an extra Chebyshev iteration)."""
    wxz, wxzUP, wyz, wyzUP, diag = _planes(wx.astype(np.float64),
                                           wy.astype(np.float64))
    v = np.ones((H, W))
    v /= np.linalg.norm(v)
    for _ in range(64):
        av = diag * v
        av[:, :-1] -= wxz[:, :-1] * v[:, 1:]
        av[:, 1:] -= wxzUP[:, 1:] * v[:, :-1]
        av[:-1, :] -= wyz[:-1, :] * v[1:, :]
        av[1:, :] -= wyzUP[1:, :] * v[:-1, :]
        nrm = np.linalg.norm(av)
        v = av / nrm
    return float(nrm)


def _prep_inputs(ae, wxwy):
    """Host-side shard prep: concat per-core bt and wpack.

    wpack ships twice: cols 0..239 raw, cols 240..479 pre-scaled by
    1/theta so iteration 0's products (which use u0 = b/theta) can read
    the RHS tile directly on device.
    """
    bt_all = np.empty((NCORE * 128, FDA), np.float32)
    wpack_all = np.empty((NCORE * 64, 2 * NPK), np.float32)
    wps = []
    lam_max = 0.0
    for b in range(B):
        wp = _wpack(wxwy[b, 0], wxwy[b, 1])
        wps.append(wp)
        # diag plane is 1 + sum of the four weight planes
        inc = wp[:, 4 * 48:5 * 48] - 1.0
        gersh = 1.0 + 2.0 * float(inc.max())
        lam_max = max(lam_max, min(gersh,
                                   1.08 * _lam_est(wxwy[b, 0], wxwy[b, 1])))
    lam_max = float(np.ceil(lam_max * 2.0) / 2.0)
    theta = np.float32((lam_max + 1.0) / 2.0)
    for b in range(B):
        wp2 = np.concatenate([wps[b], wps[b] / theta], axis=1)
        for half in range(2):
            core = 2 * b + half
            bt_all[core * 128:(core + 1) * 128] = _b2core(
                ae[b, half * CPC:(half + 1) * CPC])
            wpack_all[core * 64:(core + 1) * 64] = wp2
    return bt_all, wpack_all, lam_max


def kernel(ae: np.ndarray, wxwy: np.ndarray) -> np.ndarray:
    ae = np.asarray(ae, dtype=np.float32)
    wxwy = np.asarray(wxwy, dtype=np.float32)

    bt_all, wpack_all, lam_max = _prep_inputs(ae, wxwy)
    ex = _get_exec(lam_max)

    global _LAST_BUILD
    _LAST_BUILD = (ex, bt_all, wpack_all)

    xout = np.asarray(ex(bt_all, wpack_all)).reshape(NCORE, 128, FDA)

    out = np.empty((B, C, H, W), dtype=np.float32)
    for core in range(NCORE):
        b, half = core // 2, core % 2
        out[b, half * CPC:(half + 1) * CPC] = _core2out(xout[core])
    return out
